# revision 23
# baseline (speedup 1.0000x reference)
"""DTNNStep (gnn message passing) on 8 Trainium2 NeuronCores.

Strategy (edge-parallel, per the sharding hint):
  * Edges (2M, sorted by membership_i) are sharded across 8 cores at atom
    boundaries: core c owns atoms [12500c, 12500(c+1)) and exactly the edges
    whose destination (membership_i) falls in that range.
  * Within a core, edges are split into 4 substreams by membership_j range
    so 4 substreams pack the 128-partition segmented scan; substreams are
    processed in PAIRS to fill the PE array / DVE lanes.
  * Device pipeline per 512-col chunk (2048 edges = 4 substreams x 512):
      - dh: 4 matmuls (lhsT=Wdf_aug [101,64] bf16, rhs=centered distance
        in f8e3 [101,512]) -> psdh pair tiles [128,512] (s0 at part 0-63,
        s1 at 64-127 via tile_position).  Distance is shipped as
        (d - 0.5) in float8_e3m4; 0.5*sum_k W_df[k] is folded into the
        bias row on host, so the quantization error is zero-mean.
      - ah: 2 block-diagonal matmuls (lhsT=Wcf2 [61,128]: two Wcf blocks
        + bias row, rhs=af pair tile bf16 [61,512]) -> psah [128,512].
      - ACT copies psah -> SBUF bf16 (the only PSUM exit copy).
      - DVE mult: prod = ah_sb (SBUF bf16) * psdh (PSUM f32) -> bf16.
      - fc: 2 matmuls (lhsT=Wfc2 [128,64]: block structure contracting the
        pair's two 60-row H blocks) -> psfc [128,512] (F rows at
        0-29/30-59/64-93/94-123).
      - mask: 1 matmul (lhsT=IND [4,128] indicator, rhs=mask4 [4,512]) ->
        psmask [128,512]: broadcasts 4 host-built segment-boundary mask
        rows to their 30-row blocks (replaces a 16 MB mask DMA stream).
      - ACT tanh psfc -> packed SBUF bf16.
      - DVE segmented scan: state = psmask*state + packed; per-segment
        totals appear at segment-end columns (host-known positions).
  * Host: shards/pads inputs (layout only), reads the scan output at
    segment-end columns, adds the 4 substream partials and the
    fin = atom_features - tanh((b_df * atom_hidden) @ W_fc) correction
    computed on-device in a small trailing phase.
"""

import os
import sys

for _p in ("/opt/trn_rl_repo", "/root/.axon_site/_ro/trn_rl_repo"):
    if os.path.isdir(_p) and _p not in sys.path:
        sys.path.append(_p)

import numpy as np
from ml_dtypes import bfloat16, float8_e3m4
from contextlib import ExitStack

import concourse.bass as bass
import concourse.bacc as bacc
import concourse.mybir as mybir
import concourse.tile as tile
from concourse.bass_utils import run_bass_kernel_spmd

BF16 = mybir.dt.bfloat16
F8E3 = mybir.dt.float8e3
F32 = mybir.dt.float32

# partition row-base of each substream's 30 output rows in the packed tile
ROWBASE = (0, 30, 64, 94)


class Cfg:
    def __init__(self, n_atoms=100000, n_emb=30, n_dist=100, n_hid=60,
                 n_cores=8, n_sub=4, jrange=25000, c=512, c2=500,
                 jumbo=2048, scan_gpsimd_every=0, convert_every=4,
                 heat=4, af_fp8=True):
        self.n_atoms = n_atoms
        self.n_emb = n_emb
        self.n_dist = n_dist
        self.n_hid = n_hid
        self.n_cores = n_cores
        self.n_sub = n_sub
        self.jrange = jrange
        self.c = c              # pipeline chunk columns
        self.c2 = c2            # fin-phase chunk
        self.apc = n_atoms // n_cores
        self.jumbo = jumbo      # columns per SWDGE bulk DMA (multiple of c)
        # every Nth chunk's segmented scan runs on GpSimd instead of DVE
        # (0 = all scans on DVE)
        self.scan_gpsimd_every = scan_gpsimd_every
        # every Nth chunk: ACT also copies dh out of PSUM so the DVE mult
        # runs in 2x bf16 mode (DVE<->ACT load balancing; 0 = never)
        self.convert_every = convert_every
        # dummy ldweights per chunk: keep the PE array active through short
        # dependency gaps so the HAM clock gate stays at full rate (matmuls
        # reload their own stationary weights, so clobbering is harmless)
        self.heat = heat
        self.af_fp8 = af_fp8  # ship gathered atom features in f8e3
        assert jumbo % c == 0
        assert self.apc % c2 == 0
        assert jrange * n_sub >= n_atoms


DEFAULT_CFG = Cfg()


def build_program(cfg, cap):
    """Build + compile the (SPMD-identical) Bass program for one core."""
    c = cfg.c
    assert cap % cfg.jumbo == 0
    nd1 = cfg.n_dist + 1   # dist rows + ones row
    ne1 = cfg.n_emb + 1    # emb rows + ones row (fin phase)
    H, F = cfg.n_hid, cfg.n_emb

    nc = bacc.Bacc("TRN2", target_bir_lowering=False, debug=False,
                   num_devices=cfg.n_cores, num_swdge_queues=4)

    AFDT = F8E3 if cfg.af_fp8 else BF16
    distT = nc.dram_tensor("distT", [cfg.n_sub, nd1, cap], F8E3, kind="ExternalInput").ap()
    af2 = nc.dram_tensor("af2", [2, H + 1, cap], AFDT, kind="ExternalInput").ap()
    mask4 = nc.dram_tensor("mask4", [cfg.n_sub, cap], BF16, kind="ExternalInput").ap()
    a_fT_own = nc.dram_tensor("a_fT_own", [ne1, cfg.apc], BF16, kind="ExternalInput").ap()
    a_f_own = nc.dram_tensor("a_f_own", [cfg.n_emb, cfg.apc], F32, kind="ExternalInput").ap()
    Wdf = nc.dram_tensor("Wdf", [nd1, 64], BF16, kind="ExternalInput").ap()
    Wcf2 = nc.dram_tensor("Wcf2", [H + 1, 128], BF16, kind="ExternalInput").ap()
    Wfc2 = nc.dram_tensor("Wfc2", [128, 64], BF16, kind="ExternalInput").ap()
    IND = nc.dram_tensor("IND", [cfg.n_sub, 128], BF16, kind="ExternalInput").ap()
    Wcf = nc.dram_tensor("Wcf", [ne1, H], BF16, kind="ExternalInput").ap()
    Wfc = nc.dram_tensor("Wfc", [H, 32], BF16, kind="ExternalInput").ap()
    bdf = nc.dram_tensor("bdf", [H, 1], F32, kind="ExternalInput").ap()
    scanout = nc.dram_tensor("scanout", [128, cap], BF16, kind="ExternalOutput").ap()
    fin = nc.dram_tensor("fin", [cfg.n_emb, cfg.apc], F32, kind="ExternalOutput").ap()

    with tile.TileContext(nc) as tc, ExitStack() as ctx:
        wpool = ctx.enter_context(tc.tile_pool(name="weights", bufs=1))
        wdf_sb = wpool.tile([nd1, 64], BF16)
        nc.sync.dma_start(wdf_sb[:], Wdf[:])
        wcf2_sb = wpool.tile([H + 1, 128], BF16)
        nc.sync.dma_start(wcf2_sb[:], Wcf2[:])
        wfc2_sb = wpool.tile([128, 64], BF16)
        nc.sync.dma_start(wfc2_sb[:], Wfc2[:])
        ind_sb = wpool.tile([cfg.n_sub, 128], BF16)
        nc.sync.dma_start(ind_sb[:], IND[:])
        wcf_sb = wpool.tile([ne1, H], BF16)
        nc.sync.dma_start(wcf_sb[:], Wcf[:])
        wfc_sb = wpool.tile([H, 32], BF16)
        nc.sync.dma_start(wfc_sb[:], Wfc[:])
        bdf_sb = wpool.tile([H, 1], F32)
        nc.sync.dma_start(bdf_sb[:], bdf[:])

        # ---------- edge pipeline -------------------------------------------
        jb = cfg.jumbo
        nj = cap // jb
        tpj = jb // c
        with tc.tile_pool(name="ep_d", bufs=3) as dpool, \
             tc.tile_pool(name="ep_a", bufs=3) as apool, \
             tc.tile_pool(name="ep_m4", bufs=3) as m4pool, \
             tc.tile_pool(name="ep_ah", bufs=3) as ahpool, \
             tc.tile_pool(name="ep_pr", bufs=3) as prpool, \
             tc.tile_pool(name="ep_pk", bufs=3) as pkpool, \
             tc.tile_pool(name="ep_sc", bufs=2) as spool, \
             tc.tile_pool(name="ep_psd", bufs=4, space="PSUM") as psd, \
             tc.tile_pool(name="ep_psa", bufs=2, space="PSUM") as psa, \
             tc.tile_pool(name="ep_psf", bufs=1, space="PSUM") as psf, \
             tc.tile_pool(name="ep_psm", bufs=1, space="PSUM") as psm:
            chunk_i = 0
            for j in range(nj):
                j0 = j * jb
                dj = dpool.tile([nd1, cfg.n_sub, jb], F8E3, tag="dj")
                for k in range(cfg.n_sub):
                    nc.gpsimd.dma_start(dj[:, k, :],
                                        distT.rearrange("s r c -> r s c")[:, k, j0:j0 + jb])
                aj = apool.tile([H + 1, 2, jb], AFDT, tag="aj")
                for p in range(2):
                    nc.gpsimd.dma_start(aj[:, p, :],
                                        af2.rearrange("s r c -> r s c")[:, p, j0:j0 + jb])
                m4 = m4pool.tile([cfg.n_sub, jb], BF16, tag="m4")
                nc.gpsimd.dma_start(m4[:], mask4[:, j0:j0 + jb])
                stg = spool.tile([128, jb], BF16, tag="stg")
                for tt in range(tpj):
                    c0 = tt * c
                    ce = cfg.convert_every
                    conv = ce and (chunk_i % ce == ce - 1)

                    def heater():
                        # PE-array filler: holds HAM at the full 2.4 GHz
                        # clock through short dependency stalls
                        for _ in range(cfg.heat):
                            nc.tensor.ldweights(wdf_sb[:])
                    # ---- dh: 4 matmuls into 2 pair tiles --------------------
                    psdh = []
                    for p in range(2):
                        t_ = psd.tile([128, c], F32, tag=f"psdh{p}", bufs=2)
                        for h in range(2):
                            s = 2 * p + h
                            nc.tensor.matmul(t_[64 * h:64 * h + 64, :],
                                             lhsT=wdf_sb[:],
                                             rhs=dj[:, s, c0:c0 + c],
                                             start=True, stop=True,
                                             tile_position=(0, 64 * h))
                        psdh.append(t_)
                    heater()
                    # ---- ah: 1 block-diag matmul per pair, ACT copy out -----
                    ahs = []
                    for p in range(2):
                        t_ = psa.tile([128, c], F32, tag=f"psah{p}", bufs=1)
                        nc.tensor.matmul(t_[:], lhsT=wcf2_sb[:],
                                         rhs=aj[:, p, c0:c0 + c],
                                         start=True, stop=True)
                        ah = ahpool.tile([128, c], BF16, tag=f"ah{p}", bufs=2)
                        nc.scalar.copy(ah[:], t_[:])
                        ahs.append(ah)
                    heater()
                    # ---- prod = ah * dh (DVE; on conv chunks ACT first
                    # copies dh to SBUF so the mult runs in 2x bf16 mode) -----
                    prods = []
                    for p in range(2):
                        pr = prpool.tile([128, c], BF16, tag=f"prod{p}", bufs=2)
                        if conv:
                            dh = ahpool.tile([128, c], BF16, tag=f"dh{p}", bufs=2)
                            nc.scalar.copy(dh[:], psdh[p][:])
                            nc.vector.tensor_tensor(pr[:], ahs[p][:], dh[:],
                                                    op=mybir.AluOpType.mult)
                        else:
                            nc.vector.tensor_tensor(pr[:], ahs[p][:], psdh[p][:],
                                                    op=mybir.AluOpType.mult)
                        prods.append(pr)
                    # ---- fc + mask matmuls ----------------------------------
                    pfc = psf.tile([128, c], F32, tag="psfc", bufs=1)
                    for p in range(2):
                        nc.tensor.matmul(pfc[64 * p:64 * p + 64, :],
                                         lhsT=wfc2_sb[:], rhs=prods[p][:],
                                         start=True, stop=True,
                                         tile_position=(0, 64 * p))
                    pmask = psm.tile([128, c], F32, tag="psmask", bufs=1)
                    nc.tensor.matmul(pmask[:], lhsT=ind_sb[:],
                                     rhs=m4[:, c0:c0 + c], start=True, stop=True)
                    heater()
                    # ---- tanh + segmented scan ------------------------------
                    packed = pkpool.tile([128, c], BF16, tag="packed")
                    nc.scalar.activation(packed[:], pfc[:],
                                         mybir.ActivationFunctionType.Tanh)
                    # Scans are carry-free: segments straddling a chunk
                    # boundary leave a partial total at the chunk's last
                    # column, which the host adds to the same atom.
                    ge = cfg.scan_gpsimd_every
                    seng = nc.gpsimd if (ge and chunk_i % ge == ge - 1) else nc.vector
                    seng.tensor_tensor_scan(
                        stg[:, c0:c0 + c], data0=pmask[:], data1=packed[:],
                        initial=0.0,
                        op0=mybir.AluOpType.mult, op1=mybir.AluOpType.add)
                    chunk_i += 1
                nc.sync.dma_start(scanout[:, j0:j0 + jb], stg[:])

        # ---------- fin = a_f - tanh((b_df*a_h) @ W_fc) ---------------------
        with tc.tile_pool(name="fi_in", bufs=1) as fpool, \
             tc.tile_pool(name="fi_s", bufs=3) as s2, \
             tc.tile_pool(name="fi_ps", bufs=4, space="PSUM") as p2:
            afo = fpool.tile([ne1, cfg.apc], BF16)
            nc.sync.dma_start(afo[:], a_fT_own[:])
            aff = fpool.tile([cfg.n_emb, cfg.apc], F32)
            nc.sync.dma_start(aff[:], a_f_own[:])
            for q0 in range(0, cfg.apc, cfg.c2):
                psii = p2.tile([H, cfg.c2], F32, tag="psii")
                nc.tensor.matmul(psii[:], lhsT=wcf_sb[:], rhs=afo[:, q0:q0 + cfg.c2],
                                 start=True, stop=True)
                pii = s2.tile([H, cfg.c2], BF16, tag="pii")
                nc.scalar.mul(pii[:], psii[:], bdf_sb[:, 0:1])
                psff = p2.tile([F, cfg.c2], F32, tag="psf")
                nc.tensor.matmul(psff[:], lhsT=wfc_sb[:, 0:F], rhs=pii[:],
                                 start=True, stop=True)
                th2 = s2.tile([F, cfg.c2], F32, tag="th2")
                nc.scalar.activation(th2[:], psff[:],
                                     mybir.ActivationFunctionType.Tanh)
                fn = s2.tile([F, cfg.c2], F32, tag="fn")
                nc.vector.tensor_tensor(fn[:], aff[:, q0:q0 + cfg.c2], th2[:],
                                        op=mybir.AluOpType.subtract)
                nc.sync.dma_start(fin[:, q0:q0 + cfg.c2], fn[:])

    nc.compile()
    return nc


def host_prep(inputs, cfg):
    """Shard + lay out inputs for the 8 cores. Returns (in_maps, post_data, cap)."""
    af = np.asarray(inputs["atom_features"], dtype=np.float32)
    dist = np.asarray(inputs["distance"], dtype=np.float32)
    mi = np.asarray(inputs["distance_membership_i"]).astype(np.int64)
    mj = np.asarray(inputs["distance_membership_j"]).astype(np.int64)
    W_cf = np.asarray(inputs["W_cf"], dtype=np.float32)
    W_df = np.asarray(inputs["W_df"], dtype=np.float32)
    W_fc = np.asarray(inputs["W_fc"], dtype=np.float32)
    b_cf = np.asarray(inputs["b_cf"], dtype=np.float32)
    b_df = np.asarray(inputs["b_df"], dtype=np.float32)

    n_emb, n_dist, H = cfg.n_emb, cfg.n_dist, cfg.n_hid

    # dh weights: [101, 64], cols 60-63 zero.  Distance is shipped centered
    # (d - 0.5), so fold 0.5*colsum(W_df) into the bias row.
    Wdf_aug = np.zeros((n_dist + 1, 64), np.float32)
    Wdf_aug[:n_dist, :H] = W_df
    Wdf_aug[n_dist, :H] = b_df + 0.5 * W_df.sum(axis=0)
    Wdf_aug = Wdf_aug.astype(bfloat16)

    # ah weights: block-diag [61, 128]: cols 0-59 <- rows 0-29 (s_even),
    # cols 64-123 <- rows 30-59 (s_odd); bias row 60 on both blocks.
    Wcf2 = np.zeros((H + 1, 128), np.float32)
    Wcf2[0:n_emb, 0:H] = W_cf
    Wcf2[n_emb:2 * n_emb, 64:64 + H] = W_cf
    Wcf2[H, 0:H] = b_cf
    Wcf2[H, 64:64 + H] = b_cf
    Wcf2 = Wcf2.astype(bfloat16)

    # fc weights: [128, 64]: cols 0-29 contract prod rows 0-59 (s_even),
    # cols 30-59 contract prod rows 64-123 (s_odd).
    Wfc2 = np.zeros((128, 64), np.float32)
    Wfc2[0:H, 0:n_emb] = W_fc
    Wfc2[64:64 + H, n_emb:2 * n_emb] = W_fc
    Wfc2 = Wfc2.astype(bfloat16)

    # mask broadcast indicator [4, 128]: row k -> ROWBASE[k]..+30
    INDm = np.zeros((cfg.n_sub, 128), np.float32)
    for k in range(cfg.n_sub):
        INDm[k, ROWBASE[k]:ROWBASE[k] + n_emb] = 1.0
    INDm = INDm.astype(bfloat16)

    # fin-phase weights
    Wcf_aug = np.vstack([W_cf, b_cf[None, :]]).astype(bfloat16)
    Wfc_pad = np.zeros((H, 32), np.float32)
    Wfc_pad[:, :n_emb] = W_fc
    Wfc_pad = Wfc_pad.astype(bfloat16)
    bdf_col = b_df[:, None].astype(np.float32)

    af_aug = np.concatenate([af, np.ones((cfg.n_atoms, 1), np.float32)], axis=1
                            ).astype(bfloat16)  # [n_atoms, n_emb+1]
    af_dt = float8_e3m4 if cfg.af_fp8 else bfloat16
    af_bf = af.astype(af_dt)  # [n_atoms, n_emb]

    bounds = np.searchsorted(mi, np.arange(0, cfg.n_atoms + 1, cfg.apc))
    core_sels = []
    max_n = 0
    for cid in range(cfg.n_cores):
        e0, e1 = bounds[cid], bounds[cid + 1]
        kk = mj[e0:e1] // cfg.jrange
        sels = [e0 + np.nonzero(kk == k)[0] for k in range(cfg.n_sub)]
        core_sels.append(sels)
        max_n = max(max_n, max(len(s) for s in sels))
    jb = cfg.jumbo
    cap = max(jb, ((max_n + jb - 1) // jb) * jb)

    dist_q = (dist - np.float32(0.5)).astype(float8_e3m4)  # centered e3m4

    in_maps = []
    post_data = []
    for cid in range(cfg.n_cores):
        A0 = cid * cfg.apc
        sels = core_sels[cid]
        distT = np.zeros((cfg.n_sub, n_dist + 1, cap), float8_e3m4)
        af2 = np.zeros((2, H + 1, cap), af_dt)
        mask4 = np.ones((cfg.n_sub, cap), np.float32)
        ends_k = []
        for k in range(cfg.n_sub):
            sel = sels[k]
            n = len(sel)
            if n:
                distT[k, :n_dist, :n] = dist_q[sel].T
                distT[k, n_dist, :n] = float8_e3m4(1.0)
                p, h = divmod(k, 2)
                af2[p, h * n_emb:(h + 1) * n_emb, :n] = af_bf[mj[sel]].T
                ids = mi[sel] - A0
                # segment starts: id changes + forced breaks at chunk
                # boundaries (scans are carry-free; host merges partials)
                newseg = np.zeros(n, bool)
                newseg[0] = True
                newseg[1:] = ids[1:] != ids[:-1]
                newseg[np.arange(cfg.c, n, cfg.c)] = True
                m = np.ones(cap, np.float32)
                m[:n][newseg] = 0.0
                mask4[k] = m
                endpos = np.nonzero(np.r_[newseg[1:], True])[0]
                ends_k.append((endpos.astype(np.int64), ids[endpos].astype(np.int64)))
            else:
                ends_k.append((np.zeros(0, np.int64), np.zeros(0, np.int64)))
        af2[:, H, :] = af_dt(1.0)  # ones row for the ah bias
        in_maps.append(dict(
            distT=distT,
            af2=af2,
            mask4=mask4.astype(bfloat16),
            a_fT_own=np.ascontiguousarray(af_aug[A0:A0 + cfg.apc].T),
            a_f_own=np.ascontiguousarray(af[A0:A0 + cfg.apc].T.astype(np.float32)),
            Wdf=Wdf_aug, Wcf2=Wcf2, Wfc2=Wfc2, IND=INDm,
            Wcf=Wcf_aug, Wfc=Wfc_pad, bdf=bdf_col,
        ))
        post_data.append(ends_k)
    return in_maps, post_data, cap


def host_post(results, post_data, cfg):
    out = np.empty((cfg.n_atoms, cfg.n_emb), np.float32)
    for cid in range(cfg.n_cores):
        r = results[cid]
        agg = np.asarray(r["fin"]).astype(np.float32).T.copy()  # [apc, n_emb]
        sc = np.asarray(r["scanout"])  # bf16 [128, cap]
        for k in range(cfg.n_sub):
            endpos, atoms = post_data[cid][k]
            if len(endpos):
                rb = ROWBASE[k]
                vals = sc[rb:rb + cfg.n_emb][:, endpos].astype(np.float32)
                np.add.at(agg, atoms, vals.T)
        out[cid * cfg.apc:(cid + 1) * cfg.apc] = agg
    return out


_CACHE = {}


def kernel(**inputs):
    cfg = DEFAULT_CFG
    in_maps, post_data, cap = host_prep(inputs, cfg)
    if cap not in _CACHE:
        _CACHE[cap] = build_program(cfg, cap)
    nc = _CACHE[cap]
    res = run_bass_kernel_spmd(nc, in_maps, core_ids=list(range(cfg.n_cores)))
    return host_post(res.results, post_data, cfg)
